# revision 2
# baseline (speedup 1.0000x reference)
"""Trainium2 kernel for nn_AttDecoder: attention decoder with GRU recurrence.

Strategy: data-parallel over batch across 8 NeuronCores (B=32 -> 4/core),
weights replicated, the T=48 scan recurrence stays local per core. No
collectives are needed; outputs are gathered on host.

Key optimization vs the reference graph: the coverage-attention conv
(1->512ch, 11x11) followed by the [512,512] channel projection is fused
into a single 1->512ch conv with kernel  M[a,i,j] = sum_c K[c,i,j]*W[c,a]
(exact algebra), cutting per-step FLOPs ~3x. Per-step GRU input
projections and embedding terms are hoisted out of the scan as batched
GEMMs over all T.
"""

import numpy as np

B, C, H, W = 32, 684, 16, 64
T = 48
E = 256
HID = 256
A = 512
V = 111
LOC = 432
RATIO = 16
N_CORES = 8
BL = B // N_CORES  # local batch per core

_COMPILED = {}


def _build_local_fn():
    import jax
    import jax.numpy as jnp
    from jax import lax

    def local_fn(cnn, mask2, embs, localization_pred,
                 W_init, b_init, gru_Wih, gru_Whh, gru_bih, gru_bhh,
                 hw_W, hw_b, M2k, ac_W, ac_b, enc_conv_k, enc_conv_b,
                 ws_W, ws_b, we_W, we_b, wc_W, wc_b, loc_W, loc_b,
                 out_W, out_b):
        # cnn: [BL,C,H,W]; mask2: [BL,H,W]; embs: [BL,T,E]
        msum = mask2.sum((1, 2))                                   # [BL]
        avg = (cnn * mask2[:, None]).sum((2, 3)) / msum[:, None]   # [BL,C]
        hidden0 = jnp.tanh(avg @ W_init + b_init)                  # [BL,HID]
        loc_weighted = localization_pred @ loc_W + loc_b           # [BL,HID]
        cnn_trans_p = jnp.einsum('bchw,ac->bhwa', cnn, enc_conv_k) + enc_conv_b

        # hoisted per-step input projections (parallel over T)
        gi_all = embs @ gru_Wih.T + gru_bih                        # [BL,T,3H]
        we_all = embs @ we_W + we_b                                # [BL,T,HID]
        gi_t = jnp.transpose(gi_all, (1, 0, 2))                    # [T,BL,3H]
        we_t = jnp.transpose(we_all, (1, 0, 2))                    # [T,BL,HID]

        def step(carry, xs):
            hidden, alpha_sum = carry                              # [BL,HID],[BL,1,H,W]
            gi, we_emb = xs
            gh = hidden @ gru_Whh.T + gru_bhh
            ir, iz, inn = jnp.split(gi, 3, axis=-1)
            hr, hz, hn = jnp.split(gh, 3, axis=-1)
            r = jax.nn.sigmoid(ir + hr)
            z = jax.nn.sigmoid(iz + hz)
            n = jnp.tanh(inn + r * hn)
            hidden = (1.0 - z) * n + z * hidden

            query = hidden @ hw_W + hw_b                           # [BL,A]
            cov_a = lax.conv_general_dilated(alpha_sum, M2k, (1, 1),
                                             [(5, 5), (5, 5)])     # [BL,A,H,W]
            score = jnp.tanh(query[:, None, None, :]
                             + jnp.transpose(cov_a, (0, 2, 3, 1))
                             + cnn_trans_p)                        # [BL,H,W,A]
            energy = score @ ac_W + ac_b                           # [BL,H,W,1]
            # reference subtracts the global max; softmax cancels any shift
            # (the +1e-10 epsilon effect is ~1e-10 relative), so a local max
            # keeps everything core-local.
            energy = energy - energy.max()
            e = jnp.exp(energy[..., 0]) * mask2                    # [BL,H,W]
            alpha = e / (e.sum((1, 2))[:, None, None] + 1e-10)
            amask = (alpha > 0.02).astype(alpha.dtype)
            alpha_sum = alpha_sum + alpha[:, None]
            ctx = jnp.einsum('bhw,bchw->bc', alpha * amask, cnn)   # [BL,C]
            out_state = jnp.maximum(hidden @ ws_W + ws_b + we_emb
                                    + ctx @ wc_W + wc_b, loc_weighted)
            prob = out_state @ out_W + out_b                       # [BL,V]
            return (hidden, alpha_sum), (prob, alpha)

        alpha_sum0 = jnp.zeros((cnn.shape[0], 1, H, W), cnn.dtype)
        _, (probs, alphas) = lax.scan(step, (hidden0, alpha_sum0), (gi_t, we_t))
        word_probs = jnp.transpose(probs, (1, 0, 2))               # [BL,T,V]
        word_alphas = jnp.transpose(alphas, (1, 0, 2, 3))          # [BL,T,H,W]
        return word_probs, word_alphas

    return local_fn


def kernel(cnn_features, labels, localization_pred, images_mask,
           W_init, b_init, emb_table, gru_Wih, gru_Whh, gru_bih, gru_bhh,
           hw_W, hw_b, att_conv_k, att_W, ac_W, ac_b, enc_conv_k, enc_conv_b,
           ws_W, ws_b, we_W, we_b, wc_W, wc_b, loc_W, loc_b, out_W, out_b):
    import jax

    devs = jax.devices()[:N_CORES]

    # ---- host-side prep (cheap, avoids int gathers / 32MB mask on device)
    f32 = np.float32
    cnn_features = np.asarray(cnn_features, f32)
    mask2_full = np.asarray(images_mask, f32)[:, 0, ::RATIO, ::RATIO]  # [B,H,W]
    labels = np.asarray(labels)
    tokens = np.concatenate(
        [np.ones((B, 1), labels.dtype), labels[:, :-1]], axis=1).astype(np.int64)
    embs_full = np.asarray(emb_table, f32)[tokens]                     # [B,T,E]
    # fused coverage kernel: M2k[a,0,i,j] = sum_c att_conv_k[c,0,i,j]*att_W[c,a]
    M2k = np.einsum('cij,ca->aij', np.asarray(att_conv_k, f32)[:, 0],
                    np.asarray(att_W, f32)).astype(f32)[:, None]       # [A,1,11,11]

    weights = [np.asarray(w, f32) for w in
               (W_init, b_init, gru_Wih, gru_Whh, gru_bih, gru_bhh,
                hw_W, hw_b, M2k, ac_W, ac_b, enc_conv_k, enc_conv_b,
                ws_W, ws_b, we_W, we_b, wc_W, wc_b, loc_W, loc_b,
                out_W, out_b)]

    if 'fn' not in _COMPILED:
        _COMPILED['fn'] = jax.jit(_build_local_fn())
    fn = _COMPILED['fn']

    # ---- shard + async dispatch to all 8 cores (pure data parallel)
    futs = []
    for i, d in enumerate(devs):
        sl = slice(i * BL, (i + 1) * BL)
        args = [jax.device_put(cnn_features[sl], d),
                jax.device_put(mask2_full[sl], d),
                jax.device_put(embs_full[sl], d),
                jax.device_put(np.asarray(localization_pred, f32)[sl], d)]
        args += [jax.device_put(w, d) for w in weights]
        futs.append(fn(*args))

    probs = np.concatenate([np.asarray(p) for p, _ in futs], axis=0)
    alphas = np.concatenate([np.asarray(a) for _, a in futs], axis=0)
    return probs.astype(f32), alphas.astype(f32)
